# revision 8
# baseline (speedup 1.0000x reference)
"""BinarizedFCLayer forward on 8 trn2 NeuronCores.

    out = X @ sign(W).T      X: [8192, 2048] f32, W: [2048, 2048] f32
                             sign(w) = +1 if w >= 0 else -1

Strategy
--------
Data-parallel over the batch dim of X: core c computes rows
[c*1024, (c+1)*1024) of the output; W is replicated.

Per core (M=1024, K=2048, N=2048 -> 8.6 GFLOP(MAC), ~109 us at the 78.6 TF/s
16-bit TensorE peak; 24 MiB of f32 input DMA, overlapped):
  * Strict engine separation (mixing input dma_starts into an engine stream
    that also has compute head-of-line-blocks that engine on staging-buffer
    waits), and a 1:2 W:X bandwidth split matched to the k-streamed chunk-0
    consumption ratio (0.5 MiB W + 1 MiB X reads per k-tile pair):
      - GpSimd SWDGE: even X k-tile pairs as cast-DMAs f32->f16 straight
        into the resident tile (k-major, covering all of M).
      - scalar HWDGE ring: odd X k-tile pairs as f32 into staging (DVE
        casts them); output stores follow on the same ring.
      - sync HWDGE ring: all W pieces, f32, 0.5 MiB, just-in-time order.
      - DVE: binarize W pieces f32 -> exact +-1 f16 (is_ge; *2-1) + the
        odd-pair X casts.  ACT: PSUM->SBUF copies + store issues only.
    Host pre-packs both operands [chunk, part, kt, free] so every DMA line
    is 4-8 KiB contiguous.
  * PE schedule: for each W chunk nn (2048x512), run kt-outer across ALL
    8 PSUM banks (4 m-quarters x 2 m-tiles, N=512 each), accumulating 16
    k-tiles. Chunk 0 is consumed k-tile-by-k-tile as W/X stream in -- the
    DMA ramp overlaps 27 us of real matmuls instead of one unit's 6.8 us.
    Later chunks are fully resident when reached. The last chunk runs
    m-serial (unit-major) so the final PSUM copy + 0.25 MiB store overlap
    the remaining matmuls (short kernel tail).
  * Warm-up matmuls bridge the preamble and hold the HAM clock gate.

The walrus build here allows at most ONE sync wait per instruction, so a
post-pass splits any multi-wait instruction into single-wait NoOps on the
same engine placed immediately before it.
"""

import numpy as np

try:
    import concourse.bass as bass
except ImportError:  # harness may run from a bare directory
    import sys
    for p in ("/opt/trn_rl_repo", "/root/.axon_site/_ro/trn_rl_repo"):
        if p not in sys.path:
            sys.path.append(p)
    import concourse.bass as bass

import concourse.mybir as mybir
from concourse.tile import TileContext
from concourse.bass_utils import run_bass_kernel_spmd

P = 128
N_CORES = 8
M_FULL, K, N = 8192, 2048, 2048
M = M_FULL // N_CORES          # 1024 rows of X per core
KT = K // P                    # 16 k-tiles
NCH, NW = 4, 512               # 4 n-chunks of 512 (one PSUM bank each)
MQ, MW = 4, 256                # m-quarters of 256 (2 m-tiles)
WKP = 2                        # k-tiles per W DMA piece (0.5 MiB)
XKP = 2                        # k-tiles per X DMA piece (1 MiB f32 read)
N_WARM = 96                    # dummy matmuls bridging preamble -> first data

f32 = mybir.dt.float32
f16 = mybir.dt.float16


def _split_multiwait_instructions(nc: bass.Bass) -> int:
    """walrus codegen rejects >1 sync wait per instruction. Hoist extra waits
    onto fresh single-wait NoOps on the same engine right before the
    offending instruction (same-engine sequential waits are equivalent)."""
    n_split = 0
    for fn in nc.m.functions:
        for blk in fn.blocks:
            out = []
            for inst in blk.instructions:
                si = inst.sync_info
                if si is not None and si.on_wait and len(si.on_wait) > 1:
                    waits = list(si.on_wait)
                    for j, w in enumerate(waits[:-1]):
                        nop = mybir.InstNoOp(
                            name=f"{inst.name}_wsplit{j}", ins=[], outs=[])
                        nop.engine = inst.engine
                        nop.sync_info = mybir.SyncInfo(
                            on_wait=[w], on_update=[])
                        out.append(nop)
                        n_split += 1
                    inst.sync_info = mybir.SyncInfo(
                        on_wait=[waits[-1]],
                        on_update=list(si.on_update or []))
                out.append(inst)
            blk.instructions[:] = out
    return n_split


def _build_nc() -> bass.Bass:
    nc = bass.Bass()
    # Host-packed layouts (see _run):
    #   xh[mq, p, kt, mw]: X^T quarter-major; 4 KiB contiguous per (mq,p,kq).
    #   wh[nn, p, kt, nw]: W^T chunk-major; 4 KiB contiguous per (nn,p,kp).
    xh = nc.declare_dram_parameter("xh", [MQ, P, KT, MW], f32, isOutput=False)
    wh = nc.declare_dram_parameter("wh", [NCH, P, KT, NW], f32, isOutput=False)
    out = nc.declare_dram_parameter("out", [M, N], f32, isOutput=True)

    out3 = out[:].rearrange("(mt p) n -> p mt n", p=P)  # [128, 8, 2048]
    xh_r = xh[:].rearrange("mq p kt mw -> p mq kt mw")  # [128, 4, 16, 256]

    with TileContext(nc) as tc:
        with (
            tc.tile_pool(name="resident", bufs=1) as res_pool,
            tc.tile_pool(name="wq", bufs=4) as wq_pool,
            tc.tile_pool(name="wstage", bufs=6) as ws_pool,
            tc.tile_pool(name="xstage", bufs=3) as xs_pool,
            tc.tile_pool(name="osb", bufs=6) as o_pool,
            tc.tile_pool(name="psum", bufs=8, space="PSUM") as p_pool,
            tc.tile_pool(name="warm", bufs=1) as warm_pool,
        ):
            # PE warm-up first: memset + dummy matmuls queue on PE before
            # anything else, so the HAM activity monitor un-throttles the
            # array while inputs stream in.
            wsrc = warm_pool.tile([P, P], f16, tag="wsrc", name="wsrc")
            nc.vector.memset(wsrc[:], 0.0)
            wps = p_pool.tile([P, NW], f32, tag="ps", name="wps")
            for _ in range(N_WARM):
                nc.tensor.matmul(wps[:, :P], lhsT=wsrc[:], rhs=wsrc[:],
                                 start=True, stop=True)

            # Resident 16-bit operands.
            xq = res_pool.tile([P, MQ, KT, MW], f16, tag="xq", name="xq")
            wqs = [wq_pool.tile([P, KT, NW], f16, tag="wq", name=f"wq{nn}")
                   for nn in range(NCH)]

            # X, k-major all-M pieces of XKP k-tiles: even pieces ride the
            # GpSimd SWDGE queue as cast-DMAs straight into xq; odd pieces
            # ride the scalar HWDGE ring as f32 and DVE casts them. Two
            # queues for X + one for W matches the 2:1 X:W read ratio that
            # chunk-0's k-streamed consumption needs.
            xodd = {}
            for kp in range(KT // XKP):
                ks = slice(kp * XKP, (kp + 1) * XKP)
                if kp % 2 == 0:
                    nc.gpsimd.dma_start(out=xq[:, :, ks, :],
                                        in_=xh_r[:, :, ks, :])
                else:
                    t = xs_pool.tile([P, MQ, XKP, MW], f32, tag="xs",
                                     name=f"xs{kp}")
                    nc.scalar.dma_start(out=t[:], in_=xh_r[:, :, ks, :])
                    xodd[kp] = t

            # W: plain f32 pieces on the sync HWDGE ring, just-in-time order.
            wstages = {}
            for nn in range(NCH):
                for kp in range(KT // WKP):
                    ks = slice(kp * WKP, (kp + 1) * WKP)
                    t = ws_pool.tile([P, WKP, NW], f32, tag="ws",
                                     name=f"ws{nn}_{kp}")
                    nc.sync.dma_start(out=t[:], in_=wh[nn, :, ks, :])
                    wstages[(nn, kp)] = t

            # DVE, in chunk-0 consumption order: binarize each W piece as it
            # lands (f32 -> exact +-1 f16) interleaved with the odd X casts.
            def binarize(nn, kp):
                ks = slice(kp * WKP, (kp + 1) * WKP)
                nc.vector.tensor_scalar(
                    wqs[nn][:, ks, :], wstages[(nn, kp)][:], 0.0, None,
                    mybir.AluOpType.is_ge)
                nc.vector.tensor_scalar(
                    wqs[nn][:, ks, :], wqs[nn][:, ks, :], 2.0, -1.0,
                    mybir.AluOpType.mult, mybir.AluOpType.add)

            for kp in range(KT // WKP):
                binarize(0, kp)
                if kp % 2 == 1:
                    ks = slice(kp * XKP, (kp + 1) * XKP)
                    nc.vector.tensor_copy(out=xq[:, :, ks, :],
                                          in_=xodd[kp][:])
            for nn in range(1, NCH):
                for kp in range(KT // WKP):
                    binarize(nn, kp)

            def flush(nn, mq, mo, psum):
                nc.scalar.activation(
                    out=osbs[(mq, mo)][:], in_=psum[:],
                    func=mybir.ActivationFunctionType.Copy)
                nc.scalar.dma_start(
                    out=out3[:, mq * 2 + mo, nn * NW:(nn + 1) * NW],
                    in_=osbs[(mq, mo)][:])

            # PE: per W chunk, kt-outer across all 8 PSUM banks (4 mq x 2 mo)
            # -- chunk 0 streams k-tile-by-k-tile as the inputs land. The
            # last chunk runs m-serial so its stores overlap remaining MMs.
            for nn in range(NCH):
                psums = {(mq, mo): p_pool.tile([P, NW], f32, tag="ps",
                                               name=f"ps{nn}_{mq}_{mo}")
                         for mq in range(MQ) for mo in range(2)}
                osbs = {(mq, mo): o_pool.tile([P, NW], f32, tag="osb",
                                              name=f"osb{nn}_{mq}_{mo}")
                        for mq in range(MQ) for mo in range(2)}

                def mm(kt, mq, mo):
                    nc.tensor.matmul(
                        psums[(mq, mo)][:],
                        lhsT=xq[:, mq, kt, mo * P:(mo + 1) * P],
                        rhs=wqs[nn][:, kt, :],
                        start=(kt == 0),
                        stop=(kt == KT - 1),
                    )

                if nn < NCH - 1:
                    for kt in range(KT):
                        for mq in range(MQ):
                            for mo in range(2):
                                mm(kt, mq, mo)
                    for mq in range(MQ):
                        for mo in range(2):
                            flush(nn, mq, mo, psums[(mq, mo)])
                else:
                    for mq in range(MQ):
                        for mo in range(2):
                            for kt in range(KT):
                                mm(kt, mq, mo)
                            flush(nn, mq, mo, psums[(mq, mo)])

    _split_multiwait_instructions(nc)
    return nc


_NC_CACHE = None


def _get_nc() -> bass.Bass:
    global _NC_CACHE
    if _NC_CACHE is None:
        _NC_CACHE = _build_nc()
    return _NC_CACHE


def _pack_inputs(X: np.ndarray, W: np.ndarray):
    """Host-side layout prep (pure data movement, no value changes).

    xh[c]: [MQ, P, KT, MW] with xh[c][mq, p, kt, m] = X[c*M + mq*MW + m,
                                                        kt*P + p]
    wh:    [NCH, P, KT, NW] with wh[nn, p, kt, n] = W[nn*NW + n, kt*P + p]
    """
    XT = X.T.reshape(KT, P, N_CORES, MQ, MW)        # [kt, p, c, mq, mw]
    xh = np.ascontiguousarray(XT.transpose(2, 3, 1, 0, 4))  # [c, mq, p, kt, mw]
    WT = W.T.reshape(KT, P, NCH, NW)                # [kt, p, nn, nw]
    wh = np.ascontiguousarray(WT.transpose(2, 1, 0, 3))     # [nn, p, kt, nw]
    return xh, wh


def _run(inputs: dict, trace: bool = False, **kw):
    X = np.asarray(inputs["X"], dtype=np.float32)
    W = np.asarray(inputs["W"], dtype=np.float32)
    assert X.shape == (M_FULL, K) and W.shape == (N, K)

    xh, wh = _pack_inputs(X, W)
    in_maps = [{"xh": xh[c], "wh": wh} for c in range(N_CORES)]
    res = run_bass_kernel_spmd(
        _get_nc(), in_maps, list(range(N_CORES)), trace=trace, **kw)
    out = np.concatenate([res.results[c]["out"] for c in range(N_CORES)],
                         axis=0)
    return out, res


def kernel(X: np.ndarray, W: np.ndarray) -> np.ndarray:
    out, _ = _run({"X": X, "W": W})
    return out


# revision 14
# speedup vs baseline: 1.0313x; 1.0313x over previous
"""BinarizedFCLayer forward on 8 trn2 NeuronCores.

    out = X @ sign(W).T      X: [8192, 2048] f32, W: [2048, 2048] f32
                             sign(w) = +1 if w >= 0 else -1

Strategy
--------
Data-parallel over the batch dim of X: core c computes rows
[c*1024, (c+1)*1024) of the output; W is replicated.

Per core (M=1024, K=2048, N=2048 -> 8.6 GFLOP(MAC), ~109 us at the 78.6 TF/s
16-bit TensorE peak; 24 MiB of f32 input DMA, overlapped):
  * Strict engine separation (mixing input dma_starts into an engine stream
    that also has compute head-of-line-blocks that engine on staging-buffer
    waits), and a 1:2 W:X bandwidth split matched to the k-streamed chunk-0
    consumption ratio (0.5 MiB W + 1 MiB X reads per k-tile pair):
      - GpSimd SWDGE: even X k-tile pairs as cast-DMAs f32->f16 straight
        into the resident tile (k-major, covering all of M).
      - scalar HWDGE ring: odd X k-tile pairs as f32 into staging (DVE
        casts them); output stores follow on the same ring.
      - sync HWDGE ring: all W pieces, f32, 0.5 MiB, just-in-time order.
      - DVE: binarize W pieces f32 -> exact +-1 f16 (is_ge; *2-1) + the
        odd-pair X casts.  ACT: PSUM->SBUF copies + store issues only.
    Host pre-packs both operands [chunk, part, kt, free] so every DMA line
    is 4-8 KiB contiguous.
  * PE schedule: for each W chunk nn (2048x512), run kt-outer across ALL
    8 PSUM banks (4 m-quarters x 2 m-tiles, N=512 each), accumulating 16
    k-tiles. Chunk 0 is consumed k-tile-by-k-tile as W/X stream in -- the
    DMA ramp overlaps 27 us of real matmuls instead of one unit's 6.8 us.
    Later chunks are fully resident when reached. The last chunk runs
    m-serial (unit-major) so the final PSUM copy + 0.25 MiB store overlap
    the remaining matmuls (short kernel tail).
  * Warm-up matmuls bridge the preamble and hold the HAM clock gate.

The walrus build here allows at most ONE sync wait per instruction, so a
post-pass splits any multi-wait instruction into single-wait NoOps on the
same engine placed immediately before it.
"""

import numpy as np

try:
    import concourse.bass as bass
except ImportError:  # harness may run from a bare directory
    import sys
    for p in ("/opt/trn_rl_repo", "/root/.axon_site/_ro/trn_rl_repo"):
        if p not in sys.path:
            sys.path.append(p)
    import concourse.bass as bass

import concourse.mybir as mybir
from concourse.tile import TileContext
from concourse.bass_utils import run_bass_kernel_spmd

P = 128
N_CORES = 8
M_FULL, K, N = 8192, 2048, 2048
M = M_FULL // N_CORES          # 1024 rows of X per core
KT = K // P                    # 16 k-tiles
NCH, NW = 4, 512               # 4 n-chunks of 512 (one PSUM bank each)
MQ, MW = 4, 256                # m-quarters of 256 (2 m-tiles)
WKP = 2                        # k-tiles per W DMA piece (0.5 MiB)
XKP = 2                        # k-tiles per X DMA piece (1 MiB f32 read)
N_WARM = 96                    # dummy matmuls bridging preamble -> first data

f32 = mybir.dt.float32
f16 = mybir.dt.float16


def _split_multiwait_instructions(nc: bass.Bass) -> int:
    """walrus codegen rejects >1 sync wait per instruction. Hoist extra waits
    onto fresh single-wait NoOps on the same engine right before the
    offending instruction (same-engine sequential waits are equivalent)."""
    n_split = 0
    for fn in nc.m.functions:
        for blk in fn.blocks:
            out = []
            for inst in blk.instructions:
                si = inst.sync_info
                if si is not None and si.on_wait and len(si.on_wait) > 1:
                    waits = list(si.on_wait)
                    for j, w in enumerate(waits[:-1]):
                        nop = mybir.InstNoOp(
                            name=f"{inst.name}_wsplit{j}", ins=[], outs=[])
                        nop.engine = inst.engine
                        nop.sync_info = mybir.SyncInfo(
                            on_wait=[w], on_update=[])
                        out.append(nop)
                        n_split += 1
                    inst.sync_info = mybir.SyncInfo(
                        on_wait=[waits[-1]],
                        on_update=list(si.on_update or []))
                out.append(inst)
            blk.instructions[:] = out
    return n_split


def _build_nc() -> bass.Bass:
    nc = bass.Bass()
    # Host-packed layouts (see _run):
    #   xh[p, kt, m]: X^T k-major; 8 KiB contiguous per (p, kt-pair).
    #   wh[nn, p, kt, nw]: W^T chunk-major; 4 KiB contiguous per (nn,p,kp).
    xh = nc.declare_dram_parameter("xh", [P, KT, M], f32, isOutput=False)
    wh = nc.declare_dram_parameter("wh", [NCH, P, KT, NW], f32, isOutput=False)
    out = nc.declare_dram_parameter("out", [M, N], f32, isOutput=True)

    out3 = out[:].rearrange("(mt p) n -> p mt n", p=P)  # [128, 8, 2048]
    xh3 = xh[:]                                         # [128, 16, 1024]

    with TileContext(nc) as tc:
        with (
            tc.tile_pool(name="resident", bufs=1) as res_pool,
            tc.tile_pool(name="wq", bufs=4) as wq_pool,
            tc.tile_pool(name="wstage", bufs=6) as ws_pool,
            tc.tile_pool(name="xstage", bufs=3) as xs_pool,
            tc.tile_pool(name="osb", bufs=6) as o_pool,
            tc.tile_pool(name="psum", bufs=8, space="PSUM") as p_pool,
            tc.tile_pool(name="warm", bufs=1) as warm_pool,
        ):
            # PE warm-up first: memset + dummy matmuls queue on PE before
            # anything else, so the HAM activity monitor un-throttles the
            # array while inputs stream in.
            wsrc = warm_pool.tile([P, P], f16, tag="wsrc", name="wsrc")
            nc.vector.memset(wsrc[:], 0.0)
            wps = p_pool.tile([P, NW], f32, tag="ps", name="wps")
            for _ in range(N_WARM):
                nc.tensor.matmul(wps[:, :P], lhsT=wsrc[:], rhs=wsrc[:],
                                 start=True, stop=True)

            # Resident 16-bit operands.
            xq = res_pool.tile([P, KT, M], f16, tag="xq", name="xq")
            wqs = [wq_pool.tile([P, KT, NW], f16, tag="wq", name=f"wq{nn}")
                   for nn in range(NCH)]

            # X, k-major all-M pieces of XKP k-tiles: even pieces ride the
            # GpSimd SWDGE queue as cast-DMAs straight into xq; odd pieces
            # ride the scalar HWDGE ring as f32 and DVE casts them. Two
            # queues for X + one for W matches the 2:1 X:W read ratio that
            # chunk-0's k-streamed consumption needs.
            xodd = {}
            for kp in range(KT // XKP):
                ks = slice(kp * XKP, (kp + 1) * XKP)
                if kp % 2 == 0:
                    nc.gpsimd.dma_start(out=xq[:, ks, :], in_=xh3[:, ks, :])
                else:
                    t = xs_pool.tile([P, XKP, M], f32, tag="xs",
                                     name=f"xs{kp}")
                    nc.scalar.dma_start(out=t[:], in_=xh3[:, ks, :])
                    xodd[kp] = t

            # W: plain f32 pieces on the sync HWDGE ring, just-in-time order.
            wstages = {}
            for nn in range(NCH):
                for kp in range(KT // WKP):
                    ks = slice(kp * WKP, (kp + 1) * WKP)
                    t = ws_pool.tile([P, WKP, NW], f32, tag="ws",
                                     name=f"ws{nn}_{kp}")
                    nc.sync.dma_start(out=t[:], in_=wh[nn, :, ks, :])
                    wstages[(nn, kp)] = t

            # DVE, in chunk-0 consumption order: binarize each W piece as it
            # lands (f32 -> exact +-1 f16) interleaved with the odd X casts.
            def binarize(nn, kp):
                ks = slice(kp * WKP, (kp + 1) * WKP)
                nc.vector.tensor_scalar(
                    wqs[nn][:, ks, :], wstages[(nn, kp)][:], 0.0, None,
                    mybir.AluOpType.is_ge)
                nc.vector.tensor_scalar(
                    wqs[nn][:, ks, :], wqs[nn][:, ks, :], 2.0, -1.0,
                    mybir.AluOpType.mult, mybir.AluOpType.add)

            for kp in range(KT // WKP):
                binarize(0, kp)
                if kp % 2 == 1:
                    ks = slice(kp * XKP, (kp + 1) * XKP)
                    nc.vector.tensor_copy(out=xq[:, ks, :], in_=xodd[kp][:])
            for nn in range(1, NCH):
                for kp in range(KT // WKP):
                    binarize(nn, kp)

            def flush(nn, mq, mo, psum):
                nc.scalar.activation(
                    out=osbs[(mq, mo)][:], in_=psum[:],
                    func=mybir.ActivationFunctionType.Copy)
                nc.scalar.dma_start(
                    out=out3[:, mq * 2 + mo, nn * NW:(nn + 1) * NW],
                    in_=osbs[(mq, mo)][:])

            # PE: per W chunk, kt-outer across all 8 PSUM banks (4 mq x 2 mo)
            # -- chunk 0 streams k-tile-by-k-tile as the inputs land. The
            # last chunk runs m-serial so its stores overlap remaining MMs.
            for nn in range(NCH):
                psums = {(mq, mo): p_pool.tile([P, NW], f32, tag="ps",
                                               name=f"ps{nn}_{mq}_{mo}")
                         for mq in range(MQ) for mo in range(2)}
                osbs = {(mq, mo): o_pool.tile([P, NW], f32, tag="osb",
                                              name=f"osb{nn}_{mq}_{mo}")
                        for mq in range(MQ) for mo in range(2)}

                def mm(kt, mq, mo):
                    mcol = mq * MW + mo * P
                    nc.tensor.matmul(
                        psums[(mq, mo)][:],
                        lhsT=xq[:, kt, mcol:mcol + P],
                        rhs=wqs[nn][:, kt, :],
                        start=(kt == 0),
                        stop=(kt == KT - 1),
                    )

                if nn < NCH - 1:
                    for kt in range(KT):
                        for mq in range(MQ):
                            for mo in range(2):
                                mm(kt, mq, mo)
                    for mq in range(MQ):
                        for mo in range(2):
                            flush(nn, mq, mo, psums[(mq, mo)])
                else:
                    for mq in range(MQ):
                        for mo in range(2):
                            for kt in range(KT):
                                mm(kt, mq, mo)
                            flush(nn, mq, mo, psums[(mq, mo)])

    _split_multiwait_instructions(nc)
    return nc


_NC_CACHE = None


def _get_nc() -> bass.Bass:
    global _NC_CACHE
    if _NC_CACHE is None:
        _NC_CACHE = _build_nc()
    return _NC_CACHE


def _pack_inputs(X: np.ndarray, W: np.ndarray):
    """Host-side layout prep (pure data movement, no value changes).

    xh[c]: [P, KT, M] with xh[c][p, kt, m] = X[c*M + m, kt*P + p]
    wh:    [NCH, P, KT, NW] with wh[nn, p, kt, n] = W[nn*NW + n, kt*P + p]
    """
    XT = X.T.reshape(KT, P, N_CORES, M)             # [kt, p, c, m]
    xh = np.ascontiguousarray(XT.transpose(2, 1, 0, 3))     # [c, p, kt, m]
    WT = W.T.reshape(KT, P, NCH, NW)                # [kt, p, nn, nw]
    wh = np.ascontiguousarray(WT.transpose(2, 1, 0, 3))     # [nn, p, kt, nw]
    return xh, wh


def _run(inputs: dict, trace: bool = False, **kw):
    X = np.asarray(inputs["X"], dtype=np.float32)
    W = np.asarray(inputs["W"], dtype=np.float32)
    assert X.shape == (M_FULL, K) and W.shape == (N, K)

    xh, wh = _pack_inputs(X, W)
    in_maps = [{"xh": xh[c], "wh": wh} for c in range(N_CORES)]
    res = run_bass_kernel_spmd(
        _get_nc(), in_maps, list(range(N_CORES)), trace=trace, **kw)
    out = np.concatenate([res.results[c]["out"] for c in range(N_CORES)],
                         axis=0)
    return out, res


def kernel(X: np.ndarray, W: np.ndarray) -> np.ndarray:
    out, _ = _run({"X": X, "W": W})
    return out


# revision 17
# speedup vs baseline: 1.0520x; 1.0201x over previous
"""BinarizedFCLayer forward on 8 trn2 NeuronCores.

    out = X @ sign(W).T      X: [8192, 2048] f32, W: [2048, 2048] f32
                             sign(w) = +1 if w >= 0 else -1

Strategy
--------
Data-parallel over the batch dim of X: core c computes rows
[c*1024, (c+1)*1024) of the output; W is replicated.

Per core (M=1024, K=2048, N=2048 -> 8.6 GFLOP(MAC), ~109 us at the 78.6 TF/s
16-bit TensorE peak; 24 MiB of f32 input DMA, overlapped):
  * Both inputs stream as plain f32 over the two HWDGE rings (the GpSimd
    SWDGE queue measured only ~150 GB/s — not used). Each ring's FIFO list
    is byte-balanced and need-ordered for the k-streamed chunk-0
    consumption ratio (0.5 MiB W + 1 MiB X reads per k-tile pair):
      - sync ring:   all W pieces (0.5 MiB) + X k-tile pairs 3 and 7.
      - scalar ring: X k-tiles 0, 1 (singles, for a fast first matmul),
        pairs 1, 2, 4, 5, 6; output stores follow once X input is done.
      - DVE: binarize W pieces f32 -> exact +-1 f16 (is_ge; *2-1) + all
        X casts f32->f16.  ACT: PSUM->SBUF copies + store issues only.
    Separate staging pools per ring so neither ring's head-of-line wait
    can couple to the other. Host pre-packs both operands so every DMA
    line is 4-8 KiB contiguous.
  * PE schedule: for each W chunk nn (2048x512), run kt-outer across ALL
    8 PSUM banks (4 m-quarters x 2 m-tiles, N=512 each), accumulating 16
    k-tiles. Chunk 0 is consumed k-tile-by-k-tile as W/X stream in -- the
    DMA ramp overlaps 27 us of real matmuls instead of one unit's 6.8 us.
    Later chunks are fully resident when reached. The last chunk runs
    m-serial (unit-major) so the final PSUM copy + 0.25 MiB store overlap
    the remaining matmuls (short kernel tail).
  * Warm-up matmuls bridge the preamble and hold the HAM clock gate.

The walrus build here allows at most ONE sync wait per instruction, so a
post-pass splits any multi-wait instruction into single-wait NoOps on the
same engine placed immediately before it.
"""

import numpy as np

try:
    import concourse.bass as bass
except ImportError:  # harness may run from a bare directory
    import sys
    for p in ("/opt/trn_rl_repo", "/root/.axon_site/_ro/trn_rl_repo"):
        if p not in sys.path:
            sys.path.append(p)
    import concourse.bass as bass

import concourse.mybir as mybir
from concourse.tile import TileContext
from concourse.bass_utils import run_bass_kernel_spmd

P = 128
N_CORES = 8
M_FULL, K, N = 8192, 2048, 2048
M = M_FULL // N_CORES          # 1024 rows of X per core
KT = K // P                    # 16 k-tiles
NCH, NW = 4, 512               # 4 n-chunks of 512 (one PSUM bank each)
MQ, MW = 4, 256                # m-quarters of 256 (2 m-tiles)
WKP = 2                        # k-tiles per W DMA piece (0.5 MiB)
XKP = 2                        # k-tiles per X DMA piece (1 MiB f32 read)
N_WARM = 96                    # dummy matmuls bridging preamble -> first data

f32 = mybir.dt.float32
f16 = mybir.dt.float16


def _split_multiwait_instructions(nc: bass.Bass) -> int:
    """walrus codegen rejects >1 sync wait per instruction. Hoist extra waits
    onto fresh single-wait NoOps on the same engine right before the
    offending instruction (same-engine sequential waits are equivalent)."""
    n_split = 0
    for fn in nc.m.functions:
        for blk in fn.blocks:
            out = []
            for inst in blk.instructions:
                si = inst.sync_info
                if si is not None and si.on_wait and len(si.on_wait) > 1:
                    waits = list(si.on_wait)
                    for j, w in enumerate(waits[:-1]):
                        nop = mybir.InstNoOp(
                            name=f"{inst.name}_wsplit{j}", ins=[], outs=[])
                        nop.engine = inst.engine
                        nop.sync_info = mybir.SyncInfo(
                            on_wait=[w], on_update=[])
                        out.append(nop)
                        n_split += 1
                    inst.sync_info = mybir.SyncInfo(
                        on_wait=[waits[-1]],
                        on_update=list(si.on_update or []))
                out.append(inst)
            blk.instructions[:] = out
    return n_split


def _build_nc() -> bass.Bass:
    nc = bass.Bass()
    # Host-packed layouts (see _run):
    #   xh[p, kt, m]: X^T k-major; 8 KiB contiguous per (p, kt-pair).
    #   wh[nn, p, kt, nw]: W^T chunk-major; 4 KiB contiguous per (nn,p,kp).
    xh = nc.declare_dram_parameter("xh", [P, KT, M], f32, isOutput=False)
    wh = nc.declare_dram_parameter("wh", [NCH, P, KT, NW], f32, isOutput=False)
    out = nc.declare_dram_parameter("out", [M, N], f32, isOutput=True)

    out3 = out[:].rearrange("(mt p) n -> p mt n", p=P)  # [128, 8, 2048]
    xh3 = xh[:]                                         # [128, 16, 1024]

    with TileContext(nc) as tc:
        with (
            tc.tile_pool(name="resident", bufs=1) as res_pool,
            tc.tile_pool(name="wq", bufs=4) as wq_pool,
            tc.tile_pool(name="wstage", bufs=6) as ws_pool,
            tc.tile_pool(name="xstageA", bufs=2) as xsa_pool,
            tc.tile_pool(name="xstageB", bufs=3) as xsb_pool,
            tc.tile_pool(name="xstage1", bufs=2) as xs1_pool,
            tc.tile_pool(name="osb", bufs=6) as o_pool,
            tc.tile_pool(name="psum", bufs=8, space="PSUM") as p_pool,
            tc.tile_pool(name="warm", bufs=1) as warm_pool,
        ):
            # PE warm-up first: memset + dummy matmuls queue on PE before
            # anything else, so the HAM activity monitor un-throttles the
            # array while inputs stream in.
            wsrc = warm_pool.tile([P, P], f16, tag="wsrc", name="wsrc")
            nc.vector.memset(wsrc[:], 0.0)
            wps = p_pool.tile([P, NW], f32, tag="ps", name="wps")
            for _ in range(N_WARM):
                nc.tensor.matmul(wps[:, :P], lhsT=wsrc[:], rhs=wsrc[:],
                                 start=True, stop=True)

            # Resident 16-bit operands.
            xq = res_pool.tile([P, KT, M], f16, tag="xq", name="xq")
            wqs = [wq_pool.tile([P, KT, NW], f16, tag="wq", name=f"wq{nn}")
                   for nn in range(NCH)]

            # X pieces, k-major covering all of M, staged f32 then DVE-cast.
            # kt 0 and 1 go as singles for a fast first matmul; pairs 3 and
            # 7 ride the sync ring (byte-balancing it against W), the rest
            # ride the scalar ring. Emission in need order per ring.
            xst = {}

            def xdma(eng, pool, tag, k0, nkt):
                t = pool.tile([P, nkt, M], f32, tag=tag, name=f"xs{k0}")
                eng.dma_start(out=t[:], in_=xh3[:, k0:k0 + nkt, :])
                xst[k0] = (t, nkt)

            wstages = {}

            def wdma(nn, kp):
                ks = slice(kp * WKP, (kp + 1) * WKP)
                t = ws_pool.tile([P, WKP, NW], f32, tag="ws",
                                 name=f"ws{nn}_{kp}")
                nc.sync.dma_start(out=t[:], in_=wh[nn, :, ks, :])
                wstages[(nn, kp)] = t

            # scalar ring: X singles kt0, kt1 then pairs (2,3),(4,5),(8,9),
            # (10,11),(12,13) -- need-ordered.
            xdma(nc.scalar, xs1_pool, "xs1", 0, 1)
            xdma(nc.scalar, xs1_pool, "xs1", 1, 1)
            for k0 in (2, 4, 8, 10, 12):
                xdma(nc.scalar, xsb_pool, "xsB", k0, 2)
            # sync ring: W chunk 0 + X pairs (6,7),(14,15) merged by need,
            # then W chunks 1-3.
            for kp in range(4):
                wdma(0, kp)
            xdma(nc.sync, xsa_pool, "xsA", 6, 2)
            for kp in range(4, 8):
                wdma(0, kp)
            xdma(nc.sync, xsa_pool, "xsA", 14, 2)
            for nn in range(1, NCH):
                for kp in range(KT // WKP):
                    wdma(nn, kp)

            # DVE, in chunk-0 consumption order: binarize each W piece as it
            # lands (f32 -> exact +-1 f16) interleaved with the X casts.
            def binarize(nn, kp):
                ks = slice(kp * WKP, (kp + 1) * WKP)
                nc.vector.tensor_scalar(
                    wqs[nn][:, ks, :], wstages[(nn, kp)][:], 0.0, None,
                    mybir.AluOpType.is_ge)
                nc.vector.tensor_scalar(
                    wqs[nn][:, ks, :], wqs[nn][:, ks, :], 2.0, -1.0,
                    mybir.AluOpType.mult, mybir.AluOpType.add)

            def xcast(k0):
                t, nkt = xst[k0]
                nc.vector.tensor_copy(out=xq[:, k0:k0 + nkt, :], in_=t[:])

            binarize(0, 0); xcast(0); xcast(1)
            binarize(0, 1); xcast(2)
            binarize(0, 2); xcast(4)
            binarize(0, 3); xcast(6)
            binarize(0, 4); xcast(8)
            binarize(0, 5); xcast(10)
            binarize(0, 6); xcast(12)
            binarize(0, 7); xcast(14)
            for nn in range(1, NCH):
                for kp in range(KT // WKP):
                    binarize(nn, kp)

            def flush(nn, mq, mo, psum):
                nc.scalar.activation(
                    out=osbs[(mq, mo)][:], in_=psum[:],
                    func=mybir.ActivationFunctionType.Copy)
                nc.scalar.dma_start(
                    out=out3[:, mq * 2 + mo, nn * NW:(nn + 1) * NW],
                    in_=osbs[(mq, mo)][:])

            # PE: per W chunk, kt-outer across all 8 PSUM banks (4 mq x 2 mo)
            # -- chunk 0 streams k-tile-by-k-tile as the inputs land. The
            # last chunk runs m-serial so its stores overlap remaining MMs.
            for nn in range(NCH):
                psums = {(mq, mo): p_pool.tile([P, NW], f32, tag="ps",
                                               name=f"ps{nn}_{mq}_{mo}")
                         for mq in range(MQ) for mo in range(2)}
                osbs = {(mq, mo): o_pool.tile([P, NW], f32, tag="osb",
                                              name=f"osb{nn}_{mq}_{mo}")
                        for mq in range(MQ) for mo in range(2)}

                def mm(kt, mq, mo):
                    mcol = mq * MW + mo * P
                    nc.tensor.matmul(
                        psums[(mq, mo)][:],
                        lhsT=xq[:, kt, mcol:mcol + P],
                        rhs=wqs[nn][:, kt, :],
                        start=(kt == 0),
                        stop=(kt == KT - 1),
                    )

                if nn < NCH - 1:
                    for kt in range(KT):
                        for mq in range(MQ):
                            for mo in range(2):
                                mm(kt, mq, mo)
                    for mq in range(MQ):
                        for mo in range(2):
                            flush(nn, mq, mo, psums[(mq, mo)])
                else:
                    for mq in range(MQ):
                        for mo in range(2):
                            for kt in range(KT):
                                mm(kt, mq, mo)
                            flush(nn, mq, mo, psums[(mq, mo)])

    _split_multiwait_instructions(nc)
    return nc


_NC_CACHE = None


def _get_nc() -> bass.Bass:
    global _NC_CACHE
    if _NC_CACHE is None:
        _NC_CACHE = _build_nc()
    return _NC_CACHE


def _pack_inputs(X: np.ndarray, W: np.ndarray):
    """Host-side layout prep (pure data movement, no value changes).

    xh[c]: [P, KT, M] with xh[c][p, kt, m] = X[c*M + m, kt*P + p]
    wh:    [NCH, P, KT, NW] with wh[nn, p, kt, n] = W[nn*NW + n, kt*P + p]
    """
    XT = X.T.reshape(KT, P, N_CORES, M)             # [kt, p, c, m]
    xh = np.ascontiguousarray(XT.transpose(2, 1, 0, 3))     # [c, p, kt, m]
    WT = W.T.reshape(KT, P, NCH, NW)                # [kt, p, nn, nw]
    wh = np.ascontiguousarray(WT.transpose(2, 1, 0, 3))     # [nn, p, kt, nw]
    return xh, wh


def _run(inputs: dict, trace: bool = False, **kw):
    X = np.asarray(inputs["X"], dtype=np.float32)
    W = np.asarray(inputs["W"], dtype=np.float32)
    assert X.shape == (M_FULL, K) and W.shape == (N, K)

    xh, wh = _pack_inputs(X, W)
    in_maps = [{"xh": xh[c], "wh": wh} for c in range(N_CORES)]
    res = run_bass_kernel_spmd(
        _get_nc(), in_maps, list(range(N_CORES)), trace=trace, **kw)
    out = np.concatenate([res.results[c]["out"] for c in range(N_CORES)],
                         axis=0)
    return out, res


def kernel(X: np.ndarray, W: np.ndarray) -> np.ndarray:
    out, _ = _run({"X": X, "W": W})
    return out
